# revision 1
# baseline (speedup 1.0000x reference)
"""Trainium2 Bass kernel for the UR5e reflected-mass cost function.

Math (per sample n of 131072 = 2048 b x 64 h):
  q = x[b,h,6:12], hand = x[b,h,19:22]
  FK chain (6 DH joints) -> frame origins p_0..p_6, z-axes z_0..z_6
  J[i,j] = z_j x (p_{i+1} - p_j)  (j<=i)        geometric Jacobian columns
  M = sum_i m_i J_i^T J_i + 0.1 I               6x6 SPD mass matrix
  d = hand - p_6 ; vd_j = J[5,j] . d
  s = vd^T M^-1 vd = |L^-1 vd|^2  (M = L L^T Cholesky, forward-solve only)
  cost = |d|^2 / s ;  out[b] = -sum_h cost

Implementation: every per-sample scalar is a [128,128] f32 SBUF tile
(16384 samples per core, 8 cores data-parallel over b).  The whole
computation is built as a symbolic scalar DAG with CSE + constant
folding, then emitted as DVE/ACT instructions balanced across engines
via the Tile framework.
"""

import math
import numpy as np

# ----------------------------------------------------------------------------
# symbolic scalar DAG
# ----------------------------------------------------------------------------

PI = math.pi
DH_A = [0.0, -0.425, -0.3922, 0.0, 0.0, 0.0]
DH_D = [0.1625, 0.0, 0.0, 0.1333, 0.0997, 0.0996]
# exact integer cos/sin of the alpha angles [pi/2, 0, 0, pi/2, -pi/2, 0]
CA = [0, 1, 1, 0, 0, 1]
SA = [1, 0, 0, 1, -1, 0]
MASS = [3.761, 8.058, 2.846, 1.37, 1.3, 0.365]
ROTOR = 0.1


class Expr:
    __slots__ = ("op", "args", "c", "id", "users", "engine", "fused_into",
                 "slot", "order")

    def __init__(self, op, args=(), c=None, i=0):
        self.op = op
        self.args = args
        self.c = c
        self.id = i
        self.users = []          # list of consumer Exprs
        self.engine = None       # 'dve' | 'act' | 'gps' | None(folded)
        self.fused_into = None   # consumer that absorbed this node
        self.slot = None
        self.order = None


class Graph:
    def __init__(self):
        self.nodes = []
        self.cse = {}

    def _mk(self, op, args=(), c=None):
        key = (op, tuple(a.id for a in args), c)
        n = self.cse.get(key)
        if n is None:
            n = Expr(op, args, c, len(self.nodes))
            self.nodes.append(n)
            self.cse[key] = n
        return n

    # ---- builders with simplification ----
    def C(self, v):
        return self._mk("const", c=float(v))

    def IN(self, ch):
        return self._mk("in", c=ch)

    def add(self, x, y):
        if x.op == "const" and y.op == "const":
            return self.C(x.c + y.c)
        if x.op == "const":
            x, y = y, x
        if y.op == "const":
            if y.c == 0.0:
                return x
            return self._mk("cadd", (x,), y.c)
        if x.op == "cmul" and x.c == -1.0:
            return self.sub(y, x.args[0])
        if y.op == "cmul" and y.c == -1.0:
            return self.sub(x, y.args[0])
        a, b = (x, y) if x.id <= y.id else (y, x)
        return self._mk("add", (a, b))

    def sub(self, x, y):
        if x.op == "const" and y.op == "const":
            return self.C(x.c - y.c)
        if y.op == "const":
            if y.c == 0.0:
                return x
            return self._mk("cadd", (x,), -y.c)
        if y.op == "cmul" and y.c == -1.0:
            return self.add(x, y.args[0])
        if x.op == "const" and x.c == 0.0:
            return self.cmul(-1.0, y)
        if x is y:
            return self.C(0.0)
        return self._mk("sub", (x, y))

    def cmul(self, c, x):
        c = float(c)
        if x.op == "const":
            return self.C(c * x.c)
        if c == 0.0:
            return self.C(0.0)
        if c == 1.0:
            return x
        if x.op == "cmul":
            return self.cmul(c * x.c, x.args[0])
        return self._mk("cmul", (x,), c)

    def mul(self, x, y):
        if x.op == "const":
            return self.cmul(x.c, y)
        if y.op == "const":
            return self.cmul(y.c, x)
        if x.op == "cmul" and y.op == "cmul":
            return self.cmul(x.c * y.c, self.mul(x.args[0], y.args[0]))
        if x.op == "cmul":
            return self.cmul(x.c, self.mul(x.args[0], y))
        if y.op == "cmul":
            return self.cmul(y.c, self.mul(x, y.args[0]))
        if x is y:
            return self._mk("square", (x,))
        a, b = (x, y) if x.id <= y.id else (y, x)
        return self._mk("mul", (a, b))

    def sinsb(self, x, scale, bias):
        # sin(scale*x + bias)
        return self._mk("sin", (x,), (float(scale), float(bias)))

    def ts2(self, x, s1, op0, s2, op1):
        # (x op0 s1) op1 s2  — one DVE tensor_scalar with two fused scalar ops
        return self._mk("ts2", (x,), (float(s1), op0, float(s2), op1))

    def trig(self, q, phase):
        """sin(q + phase) with range reduction to [-pi,pi): HW Sin LUT is
        only accurate near the principal range.  k = round((q+phase)/2pi)
        via the float magic-number trick; r0 = q - 2pi*k; sin(r0 + phase)
        with phase as activation bias."""
        MAGIC = 12582912.0  # 1.5 * 2**23: adding then subtracting rounds f32
        inv2pi = 1.0 / (2.0 * PI)
        if phase == 0.0:
            t1 = self.ts2(q, inv2pi, "mult", MAGIC, "add")
            k = self._mk("cadd", (t1,), -MAGIC)
        else:
            # phase/2pi must be added BEFORE the magic add (it would be
            # absorbed: ulp(MAGIC) = 1.0)
            t0 = self.ts2(q, inv2pi, "mult", phase * inv2pi, "add")
            t1 = self._mk("cadd", (t0,), MAGIC)
            k = self._mk("cadd", (t1,), -MAGIC)
        r0 = self.add(self.cmul(-2.0 * PI, k), q)  # fuses to one STT
        return self._mk("sin", (r0,), (1.0, float(phase)))

    def sqrt_(self, x):
        return self._mk("sqrt", (x,))

    def recip(self, x):
        return self._mk("recip", (x,))

    def dot3(self, u, v):
        t = [self.mul(u[i], v[i]) for i in range(3)]
        return self.add(self.add(t[0], t[1]), t[2])

    def cross(self, a, b):
        return [
            self.sub(self.mul(a[1], b[2]), self.mul(a[2], b[1])),
            self.sub(self.mul(a[2], b[0]), self.mul(a[0], b[2])),
            self.sub(self.mul(a[0], b[1]), self.mul(a[1], b[0])),
        ]


def build_graph():
    """Returns (graph, cost_neg_node). cost_neg = -cost per sample."""
    g = Graph()
    q = [g.IN(6 + i) for i in range(6)]
    hand = [g.IN(19 + c) for c in range(3)]
    s = [g.trig(q[i], 0.0) for i in range(6)]
    c_ = [g.trig(q[i], PI / 2) for i in range(6)]  # cos

    one, zero = g.C(1.0), g.C(0.0)
    R = [[one, zero, zero], [zero, one, zero], [zero, zero, one]]
    p = [zero, zero, zero]
    ps = [list(p)]
    zs = [[zero, zero, one]]
    for i in range(6):
        ct, st = c_[i], s[i]
        ca, sa = g.C(CA[i]), g.C(SA[i])
        # DH rotation columns
        col = [
            [ct, st, zero],
            [g.cmul(-CA[i], st) if CA[i] else zero,
             g.cmul(CA[i], ct) if CA[i] else zero, sa],
            [g.cmul(SA[i], st) if SA[i] else zero,
             g.cmul(-SA[i], ct) if SA[i] else zero, ca],
        ]
        dp = [g.cmul(DH_A[i], ct), g.cmul(DH_A[i], st), g.C(DH_D[i])]
        Rn = [[g.dot3(R[r], col[cc]) for cc in range(3)] for r in range(3)]
        pn = [g.add(p[r], g.dot3(R[r], dp)) for r in range(3)]
        R, p = Rn, pn
        ps.append(list(p))
        zs.append([R[r][2] for r in range(3)])

    # Jacobian columns J[(i,j)] = z_j x (p_{i+1} - p_j), j<=i
    J = {}
    for i in range(6):
        for j in range(i + 1):
            dif = [g.sub(ps[i + 1][cc], ps[j][cc]) for cc in range(3)]
            J[(i, j)] = g.cross(zs[j], dif)

    # mass matrix upper triangle
    M = {}
    for jj in range(6):
        for kk in range(jj, 6):
            acc = None
            for i in range(kk, 6):
                d3 = g.cmul(MASS[i], g.dot3(J[(i, jj)], J[(i, kk)]))
                acc = d3 if acc is None else g.add(acc, d3)
            if jj == kk:
                acc = g.add(acc, g.C(ROTOR))
            M[(jj, kk)] = acc

    # Cholesky M = L L^T ; keep rinv_j = 1/L_jj
    L = {}
    rinv = []
    for jc in range(6):
        dd = M[(jc, jc)]
        for t in range(jc):
            dd = g.sub(dd, g.mul(L[(jc, t)], L[(jc, t)]))
        r = g.recip(g.sqrt_(dd))
        rinv.append(r)
        for kk in range(jc + 1, 6):
            a = M[(jc, kk)]
            for t in range(jc):
                a = g.sub(a, g.mul(L[(kk, t)], L[(jc, t)]))
            L[(kk, jc)] = g.mul(a, r)

    # direction to hand, squared distance
    d = [g.sub(hand[cc], ps[6][cc]) for cc in range(3)]
    n2 = g.dot3(d, d)
    # vd = Je^T d  (Je columns are J[(5,j)])
    vd = [g.dot3(J[(5, j)], d) for j in range(6)]
    # forward solve L y = vd ; s = |y|^2
    y = []
    for j in range(6):
        a = vd[j]
        for t in range(j):
            a = g.sub(a, g.mul(L[(j, t)], y[t]))
        y.append(g.mul(a, rinv[j]))
    sacc = None
    for j in range(6):
        t = g.mul(y[j], y[j])
        sacc = t if sacc is None else g.add(sacc, t)
    # cost_neg = -n2 / s
    cost_neg = g.mul(g.cmul(-1.0, g.recip(sacc)), n2)
    return g, cost_neg


# ----------------------------------------------------------------------------
# numpy evaluation of the DAG (for validation in test.py)
# ----------------------------------------------------------------------------

def eval_numpy(g, root, chans):
    """chans: dict ch -> np array [N]. Evaluates all nodes; returns root val."""
    val = {}
    for n in g.nodes:
        if n.op == "const":
            val[n.id] = np.float32(n.c)
        elif n.op == "in":
            val[n.id] = chans[n.c]
        elif n.op == "add":
            val[n.id] = val[n.args[0].id] + val[n.args[1].id]
        elif n.op == "sub":
            val[n.id] = val[n.args[0].id] - val[n.args[1].id]
        elif n.op == "mul":
            val[n.id] = val[n.args[0].id] * val[n.args[1].id]
        elif n.op == "square":
            val[n.id] = val[n.args[0].id] * val[n.args[0].id]
        elif n.op == "cmul":
            val[n.id] = np.float32(n.c) * val[n.args[0].id]
        elif n.op == "cadd":
            val[n.id] = val[n.args[0].id] + np.float32(n.c)
        elif n.op == "sin":
            sc, b = n.c
            val[n.id] = np.sin(np.float32(sc) * val[n.args[0].id] + np.float32(b))
        elif n.op == "ts2":
            s1, op0, s2, op1 = n.c
            v = val[n.args[0].id]
            for s_, o_ in ((s1, op0), (s2, op1)):
                if o_ == "mult":
                    v = v * np.float32(s_)
                else:
                    v = v + np.float32(s_)
            val[n.id] = v
        elif n.op == "sqrt":
            val[n.id] = np.sqrt(val[n.args[0].id])
        elif n.op == "recip":
            val[n.id] = np.float32(1.0) / val[n.args[0].id]
        else:
            raise ValueError(n.op)
        if n.op != "const":
            val[n.id] = val[n.id].astype(np.float32)
    return val[root.id]


def ref_numpy(x):
    """Full-pipeline numpy reference using the DAG; x [B,H,26] -> [B]."""
    B, H, Cc = x.shape
    N = B * H
    flat = x.reshape(N, Cc).astype(np.float32)
    g, root = build_graph()
    chans = {ch: flat[:, ch] for ch in range(Cc)}
    cn = eval_numpy(g, root, chans)
    return cn.reshape(B, H).sum(axis=1)


# ----------------------------------------------------------------------------
# planning: use counts, fusion, engine assignment, slot allocation
# ----------------------------------------------------------------------------

COST = {  # ns per [128,128] f32 op, rough model for balancing
    ("dve", "tt"): 194, ("dve", "ts"): 127, ("dve", "stt"): 194,
    ("dve", "recip"): 260,
    ("act", "act"): 293,
    ("gps", "tt"): 420,
}


def plan(g, root, gps_frac=0.0):
    """Decide per-node: fusion into STT, engine, emission kind.

    Returns ordered list of nodes to emit (others folded/fused).
    """
    # use counts over live graph (reachable from root)
    reach = set()
    stack = [root]
    while stack:
        n = stack.pop()
        if n.id in reach:
            continue
        reach.add(n.id)
        stack.extend(n.args)
    for n in g.nodes:
        n.users = []
    order = [n for n in g.nodes if n.id in reach]
    for n in order:
        for a in n.args:
            a.users.append(n)

    # fusion: add/sub(x, cmul(c,y)) -> STT ; cmul(c, mul(x,y)) -> STT;
    # cmul(c, square(x)) -> STT(x,c,mult,x,mult)
    for n in order:
        if n.op in ("add", "sub"):
            for k, a in enumerate(n.args):
                if a.op == "cmul" and len(a.users) == 1 and a.fused_into is None \
                        and a.args[0].fused_into is None \
                        and a.args[0].op not in ("const",):
                    # (y*c) op other
                    n.c = ("stt_cmul", k, a.c)
                    a.fused_into = n
                    break
        elif n.op == "cmul" and n.fused_into is None:
            a = n.args[0]
            if a.op in ("mul", "square") and len(a.users) == 1 \
                    and a.fused_into is None \
                    and all(aa.fused_into is None for aa in a.args):
                # mark: n emits as STT (x*c)*y
                a.fused_into = n

    # engine assignment: greedy balance
    load = {"dve": 0.0, "act": 0.0, "gps": 0.0}
    emit = []
    for n in order:
        if n.op in ("const", "in"):
            continue
        if n.fused_into is not None:
            continue
        if n.op in ("sin", "sqrt"):
            n.engine = "act"
            load["act"] += COST[("act", "act")]
        elif n.op == "recip":
            n.engine = "dve"
            load["dve"] += COST[("dve", "recip")]
        elif n.op in ("cadd", "ts2"):
            # ACT Identity needs a registered const AP per bias value; keep on DVE
            n.engine = "dve"
            load["dve"] += COST[("dve", "ts")]
        elif n.op == "cmul" and not (isinstance(n.c, tuple)) and \
                n.args[0].fused_into is None:
            # pure affine: cheapest on DVE ts (2x mode), but ACT if idle
            if load["act"] + COST[("act", "act")] < load["dve"] + COST[("dve", "ts")]:
                n.engine = "act"
                load["act"] += COST[("act", "act")]
            else:
                n.engine = "dve"
                load["dve"] += COST[("dve", "ts")]
        elif n.op == "square":
            if load["act"] + COST[("act", "act")] < load["dve"] + COST[("dve", "tt")]:
                n.engine = "act"
                load["act"] += COST[("act", "act")]
            else:
                n.engine = "dve"
                load["dve"] += COST[("dve", "tt")]
        else:
            # tensor-tensor style (add/sub/mul/stt-fused/cmul-of-mul)
            is_stt = (n.op in ("add", "sub") and isinstance(n.c, tuple)) or \
                (n.op == "cmul" and n.args[0].fused_into is n)
            if gps_frac > 0 and not is_stt and \
                    load["gps"] + COST[("gps", "tt")] < \
                    load["dve"] + COST[("dve", "tt")]:
                n.engine = "gps"
                load["gps"] += COST[("gps", "tt")]
            else:
                n.engine = "dve"
                load["dve"] += COST[("dve", "tt")]
        emit.append(n)

    for i, n in enumerate(emit):
        n.order = i
    return emit, load


# ----------------------------------------------------------------------------
# bass emission
# ----------------------------------------------------------------------------

NCORES = 8
B_FULL, H, CH = 2048, 64, 26
N_PER_CORE = B_FULL * H // NCORES          # 16384
P = 128
FD = N_PER_CORE // P                        # 128


def _build_bass(gps_frac=0.0, repeat=1):
    import concourse.bass as bass
    from concourse.bacc import Bacc
    import concourse.mybir as mybir
    from concourse.tile import TileContext

    f32 = mybir.dt.float32
    alu = mybir.AluOpType
    AF = mybir.ActivationFunctionType

    g, root = build_graph()
    emit, load = plan(g, root, gps_frac)

    nc = Bacc()
    # register const APs needed as activation biases (non-Copy funcs)
    for cv in (PI / 2,):
        t = nc.alloc_sbuf_tensor(f"constf32-{cv}", [128, 1], f32)
        nc.gpsimd.memset(t.ap(), cv)
        nc.const_aps.aps[(f32, float(cv))] = t.ap()
    nc.all_engine_barrier()
    xs = nc.dram_tensor("xs", (N_PER_CORE, CH), f32, kind="ExternalInput")
    out = nc.dram_tensor("out", (B_FULL // NCORES,), f32, kind="ExternalOutput")

    # liveness for slot allocation
    last_use = {}
    for n in emit:
        for a in n.args:
            if a.order is not None:
                last_use[a.id] = max(last_use.get(a.id, -1), n.order)
            # fused producer's args are read by n as well
            if a.fused_into is n:
                for aa in a.args:
                    if aa.order is not None:
                        last_use[aa.id] = max(last_use.get(aa.id, -1), n.order)
    last_use[root.id] = len(emit) + 10

    with TileContext(nc) as tc:
        with tc.tile_pool(name="vals", bufs=1) as vp:
          for _rep in range(repeat):
            stage = vp.tile([P, FD * CH], f32, tag="stage", bufs=2)
            src = xs.rearrange("(p q) c -> p (q c)", p=P)
            nc.sync.dma_start(stage[:, :], src)
            stage3 = stage.rearrange("p (q c) -> p q c", c=CH)

            from collections import deque
            free_slots = deque()
            SLACK = 64  # keep reuse distance long so WAR waits are elided
            n_slots = [0]
            node_tile = {}

            def ap_of(n):
                if n.op == "in":
                    return stage3[:, :, n.c]
                return node_tile[n.id][:, :]

            def alloc(n):
                if len(free_slots) > SLACK:
                    sl = free_slots.popleft()
                else:
                    sl = n_slots[0]
                    n_slots[0] += 1
                t = vp.tile([P, FD], f32, tag=f"s{sl}", name=f"v{n.id}", bufs=2)
                n.slot = sl
                node_tile[n.id] = t
                return t

            def release_dead(i):
                for nn in emit[:0]:
                    pass

            # precompute: nodes whose last use is at order i
            by_last = {}
            for nid, lu in last_use.items():
                by_last.setdefault(lu, []).append(nid)

            eng = {"dve": nc.vector, "act": nc.scalar, "gps": nc.gpsimd}
            ALU_OF = {"add": alu.add, "sub": alu.subtract, "mul": alu.mult}

            for n in emit:
                ot = alloc(n)[:, :]
                e = eng[n.engine]
                if n.op == "sin":
                    sc, b = n.c
                    nc.scalar.activation(ot, ap_of(n.args[0]), AF.Sin,
                                         bias=float(b), scale=float(sc))
                elif n.op == "sqrt":
                    nc.scalar.activation(ot, ap_of(n.args[0]), AF.Sqrt)
                elif n.op == "recip":
                    nc.vector.reciprocal_approx_fast(out=ot, in_=ap_of(n.args[0]))
                elif n.op == "square":
                    if n.engine == "act":
                        nc.scalar.activation(ot, ap_of(n.args[0]), AF.Square)
                    else:
                        a = ap_of(n.args[0])
                        e.tensor_tensor(ot, a, a, alu.mult)
                elif n.op == "cadd":
                    if n.engine == "act":
                        nc.scalar.add(ot, ap_of(n.args[0]), float(n.c))
                    else:
                        e.tensor_scalar_add(ot, ap_of(n.args[0]), float(n.c))
                elif n.op == "ts2":
                    s1, op0, s2, op1 = n.c
                    e.tensor_scalar(ot, ap_of(n.args[0]), float(s1), float(s2),
                                    getattr(alu, op0), getattr(alu, op1))
                elif n.op == "cmul":
                    a = n.args[0]
                    if a.fused_into is n:
                        # STT: (x * c) op y
                        if a.op == "square":
                            x = yv = a.args[0]
                        else:
                            x, yv = a.args
                        e.scalar_tensor_tensor(ot, ap_of(x), float(n.c),
                                               ap_of(yv), alu.mult, alu.mult)
                    elif n.engine == "act":
                        nc.scalar.mul(ot, ap_of(n.args[0]), float(n.c))
                    else:
                        e.tensor_scalar_mul(ot, ap_of(n.args[0]), float(n.c))
                elif n.op in ("add", "sub"):
                    if isinstance(n.c, tuple) and n.c and n.c[0] == "stt_cmul":
                        _, k, cval = n.c
                        cm = n.args[k]
                        other = n.args[1 - k]
                        x = cm.args[0]
                        if n.op == "add":
                            # (x*c) + other
                            e.scalar_tensor_tensor(ot, ap_of(x), float(cval),
                                                   ap_of(other), alu.mult, alu.add)
                        else:
                            if k == 1:
                                # other - (x*c) = (x*-c) + other
                                e.scalar_tensor_tensor(ot, ap_of(x), float(-cval),
                                                       ap_of(other), alu.mult,
                                                       alu.add)
                            else:
                                # (x*c) - other
                                e.scalar_tensor_tensor(ot, ap_of(x), float(cval),
                                                       ap_of(other), alu.mult,
                                                       alu.subtract)
                    else:
                        e.tensor_tensor(ot, ap_of(n.args[0]), ap_of(n.args[1]),
                                        ALU_OF[n.op])
                elif n.op == "mul":
                    e.tensor_tensor(ot, ap_of(n.args[0]), ap_of(n.args[1]),
                                    alu.mult)
                else:
                    raise ValueError(n.op)

                # free slots whose last use was this node
                for nid in by_last.get(n.order, []):
                    nd = g.nodes[nid]
                    if nd.slot is not None and nd.id != root.id:
                        free_slots.append(nd.slot)
                        nd.slot = None

            # epilogue: per-b sums (64-sample segments), negate already folded
            osum = vp.tile([P, 2], f32, tag="osum", bufs=2)
            croot = node_tile[root.id]
            nc.vector.tensor_reduce(osum[:, 0:1], croot[:, 0:64],
                                    mybir.AxisListType.X, alu.add)
            nc.vector.tensor_reduce(osum[:, 1:2], croot[:, 64:128],
                                    mybir.AxisListType.X, alu.add)
            nc.sync.dma_start(out.rearrange("(p j) -> p j", p=P), osum[:, :])

    # run the bacc lowering passes (register allocation, wait splitting);
    # run_bass_via_pjrt serializes nc without calling finalize()
    nc.compile()
    return nc, len(emit), load, n_slots[0]


_CACHE = {}


def kernel(x, cond, time):
    from concourse.bass_utils import run_bass_kernel_spmd

    if "nc" not in _CACHE:
        import os as _os
        nc, n_ops, load, nsl = _build_bass(gps_frac=float(_os.environ.get("KERNEL_GPS", "1.0")))
        _CACHE["nc"] = nc
    nc = _CACHE["nc"]

    xf = np.ascontiguousarray(x, dtype=np.float32).reshape(B_FULL * H, CH)
    in_maps = []
    for k in range(NCORES):
        shard = xf[k * N_PER_CORE:(k + 1) * N_PER_CORE]
        in_maps.append({"xs": np.ascontiguousarray(shard)})
    res = run_bass_kernel_spmd(nc, in_maps, core_ids=list(range(NCORES)))
    _CACHE["exec_time_ns"] = res.exec_time_ns
    _CACHE["trace"] = res.instructions_and_trace
    outs = [res.results[k]["out"] for k in range(NCORES)]
    return np.concatenate(outs).astype(np.float32)


if __name__ == "__main__":
    # quick DAG stats
    g, root = build_graph()
    emit, load = plan(g, root)
    from collections import Counter
    print("emitted ops:", len(emit))
    print(Counter((n.engine, n.op) for n in emit))
    print("load est (us):", {k: v / 1000 for k, v in load.items()})



# revision 4
# speedup vs baseline: 2.4976x; 2.4976x over previous
"""Trainium2 Bass kernel for the UR5e reflected-mass cost function.

Closed-form math (per sample n of 131072 = 2048 b x 64 h):
  The last joint (q6) never affects the output (its Jacobian column is 0),
  and in the q1-rotated "cylindrical" frame every frame origin is
  p_i = (A_i, B_i, C_i) with the z-axes {z0=ez, z1=z2=z3=(0,1,0),
  z4=(s234,0,-c234)}.  All Jacobian columns, the 5x5 mass matrix, and the
  end-effector direction reduce to ~260 scalar ops instead of the naive
  ~670 of the frame-by-frame DH chain.

Implementation: every per-sample scalar is a [128,128] f32 SBUF tile
(16384 samples per core, 8 cores data-parallel over b).  The computation
is a symbolic scalar DAG with CSE + constant folding + STT fusion,
scheduled onto the DVE/ACT/GPSIMD engines with an earliest-finish-time
list scheduler and emitted through the Tile framework.
"""

import math
import numpy as np

# ----------------------------------------------------------------------------
# constants
# ----------------------------------------------------------------------------

PI = math.pi
A2C, A3C = -0.425, -0.3922
D1, D4, D5, D6 = 0.1625, 0.1333, 0.0997, 0.0996
# LINK_MASS[i] sits at frame origin p_{i+1}; link 0 (at p1) never moves.
M1, M2, M3, M4, M5 = 8.058, 2.846, 1.37, 1.3, 0.365
M23 = M2 + M3
M45 = M4 + M5
ROTOR = 0.1
MAGIC = 12582912.0  # 1.5 * 2**23 f32 round-to-int trick

# host channel order handed to the device
# 0:q2 1:q3 2:q4 3:q1 4:q5 5:hx 6:hy 7:hz
SRC_COLS = [7, 8, 9, 6, 10, 19, 20, 21]

# ----------------------------------------------------------------------------
# symbolic scalar DAG
# ----------------------------------------------------------------------------


class Expr:
    __slots__ = ("op", "args", "c", "id", "users", "engine", "fused_into",
                 "slot", "order", "prio", "start", "finish")

    def __init__(self, op, args=(), c=None, i=0):
        self.op = op
        self.args = args
        self.c = c
        self.id = i
        self.users = []
        self.engine = None
        self.fused_into = None
        self.slot = None
        self.order = None
        self.prio = 0.0
        self.start = 0.0
        self.finish = 0.0


class Graph:
    def __init__(self):
        self.nodes = []
        self.cse = {}

    def _mk(self, op, args=(), c=None):
        key = (op, tuple(a.id for a in args), c)
        n = self.cse.get(key)
        if n is None:
            n = Expr(op, args, c, len(self.nodes))
            self.nodes.append(n)
            self.cse[key] = n
        return n

    def C(self, v):
        return self._mk("const", c=float(v))

    def IN(self, ch):
        return self._mk("in", c=ch)

    def add(self, x, y):
        if x.op == "const" and y.op == "const":
            return self.C(x.c + y.c)
        if x.op == "const":
            x, y = y, x
        if y.op == "const":
            if y.c == 0.0:
                return x
            return self._mk("cadd", (x,), y.c)
        a, b = (x, y) if x.id <= y.id else (y, x)
        return self._mk("add", (a, b))

    def sub(self, x, y):
        if x.op == "const" and y.op == "const":
            return self.C(x.c - y.c)
        if y.op == "const":
            if y.c == 0.0:
                return x
            return self._mk("cadd", (x,), -y.c)
        if x.op == "const" and x.c == 0.0:
            return self.cmul(-1.0, y)
        if x is y:
            return self.C(0.0)
        return self._mk("sub", (x, y))

    def cmul(self, c, x):
        c = float(c)
        if x.op == "const":
            return self.C(c * x.c)
        if c == 0.0:
            return self.C(0.0)
        if c == 1.0:
            return x
        if x.op == "cmul":
            return self.cmul(c * x.c, x.args[0])
        return self._mk("cmul", (x,), c)

    def mul(self, x, y):
        if x.op == "const":
            return self.cmul(x.c, y)
        if y.op == "const":
            return self.cmul(y.c, x)
        if x.op == "cmul" and y.op == "cmul":
            return self.cmul(x.c * y.c, self.mul(x.args[0], y.args[0]))
        if x.op == "cmul":
            return self.cmul(x.c, self.mul(x.args[0], y))
        if y.op == "cmul":
            return self.cmul(y.c, self.mul(x, y.args[0]))
        if x is y:
            return self._mk("square", (x,))
        a, b = (x, y) if x.id <= y.id else (y, x)
        return self._mk("mul", (a, b))

    def ts2(self, x, s1, op0, s2, op1):
        return self._mk("ts2", (x,), (float(s1), op0, float(s2), op1))

    def trig(self, q, phase):
        """sin(q + phase), range-reduced so the Sin input is in [-pi, pi]."""
        inv2pi = 1.0 / (2.0 * PI)
        if phase == 0.0:
            t1 = self.ts2(q, inv2pi, "mult", MAGIC, "add")
            k = self._mk("cadd", (t1,), -MAGIC)
        else:
            t0 = self.ts2(q, inv2pi, "mult", phase * inv2pi, "add")
            t1 = self._mk("cadd", (t0,), MAGIC)
            k = self._mk("cadd", (t1,), -MAGIC)
        r0 = self.add(self.cmul(-2.0 * PI, k), q)  # fuses to one STT
        return self._mk("sin", (r0,), (1.0, float(phase)))

    def sqrt_(self, x):
        return self._mk("sqrt", (x,))

    def recip(self, x):
        return self._mk("recip", (x,))

    def sq(self, x):
        return self._mk("square", (x,))


def build_graph():
    """Returns (graph, cost_neg_node). cost_neg = -cost per sample."""
    g = Graph()
    q2, q3, q4, q1, q5 = (g.IN(i) for i in range(5))
    hx, hy, hz = (g.IN(5 + i) for i in range(3))

    q23 = g.add(q2, q3)
    q234 = g.add(q23, q4)
    s1, c1 = g.trig(q1, 0.0), g.trig(q1, PI / 2)
    s2, c2 = g.trig(q2, 0.0), g.trig(q2, PI / 2)
    s23, c23 = g.trig(q23, 0.0), g.trig(q23, PI / 2)
    s234, c234 = g.trig(q234, 0.0), g.trig(q234, PI / 2)
    s5, c5 = g.trig(q5, 0.0), g.trig(q5, PI / 2)

    # cylindrical coordinates (relative: A1 = K1 = 0, K = C - d1)
    A2 = g.cmul(A2C, c2)
    E = g.add(A2, g.cmul(A3C, c23))
    K2 = g.cmul(A2C, s2)
    K3 = g.add(K2, g.cmul(A3C, s23))
    cc = g.mul(c234, s5)
    sc_ = g.mul(s234, s5)
    c45 = g.mul(c234, c5)
    s45 = g.mul(s234, c5)
    A5 = g.add(E, g.cmul(D5, s234))
    A6 = g.sub(A5, g.cmul(D6, cc))
    K5 = g.sub(K3, g.cmul(D5, c234))
    K6 = g.sub(K5, g.cmul(D6, sc_))
    B6 = g.ts2(c5, D6, "mult", D4, "add")   # B6 = d4 + d6*c5

    # squares
    A2s, Es, A5s, A6s = g.sq(A2), g.sq(E), g.sq(A5), g.sq(A6)
    K2s, K3s, K5s, K6s = g.sq(K2), g.sq(K3), g.sq(K5), g.sq(K6)
    B6s = g.sq(B6)

    # weighted square sums (suffix style so S45 comes free)
    SA45 = g.add(g.cmul(M5, A6s), g.cmul(M4, A5s))
    SA = g.add(g.add(SA45, g.cmul(M23, Es)), g.cmul(M1, A2s))
    SK45 = g.add(g.cmul(M5, K6s), g.cmul(M4, K5s))
    SK = g.add(g.add(SK45, g.cmul(M23, K3s)), g.cmul(M1, K2s))
    M11nr = g.add(SA, SK)
    M11 = g.add(M11nr, g.C(ROTOR))
    M00 = g.add(g.add(SA, g.cmul(M5, B6s)), g.C((M3 + M4) * D4 * D4 + ROTOR))
    S45 = g.add(SA45, SK45)

    # weighted linear sums
    WK2 = g.add(g.cmul(M4, K5), g.cmul(M5, K6))
    WK = g.add(g.cmul(M23, K3), WK2)
    WA2 = g.add(g.cmul(M4, A5), g.cmul(M5, A6))
    WA = g.add(g.cmul(M23, E), WA2)

    # M row 0 (joint 1 uses (B, A) plane)
    bk6 = g.mul(B6, K6)
    bk2 = g.mul(B6, K2)
    k63 = g.sub(K6, K3)
    M01 = g.add(g.add(g.cmul(-M3 * D4, K3), g.cmul(-M4 * D4, K5)),
                g.cmul(-M5, bk6))
    M02 = g.add(g.add(M01, g.cmul((M3 + M4) * D4, K2)), g.cmul(M5, bk2))
    M03 = g.add(g.cmul(M4 * D4 * D5, c234), g.cmul(-M5, g.mul(B6, k63)))
    as5 = g.mul(A6, s5)
    bc45 = g.mul(B6, c45)
    M04 = g.add(g.cmul(M5 * D6, as5), g.cmul(-M5 * D6, bc45))

    # M block j,k in {1,2,3}
    Q2 = g.add(A2s, K2s)
    t1 = g.mul(K2, WK)
    t2 = g.mul(A2, WA)
    u12 = g.add(t1, t2)
    M12 = g.sub(g.sub(M11nr, g.cmul(M1, Q2)), u12)
    M22 = g.add(g.add(M11, g.cmul(M23 + M45 - M1, Q2)), g.cmul(-2.0, u12))
    t3 = g.mul(K3, WK2)
    t4 = g.mul(E, WA2)
    u34 = g.add(t3, t4)
    M13 = g.sub(S45, u34)
    Q3 = g.add(K3s, Es)
    M33 = g.add(g.add(S45, g.cmul(-2.0, u34)),
                g.ts2(Q3, M45, "mult", ROTOR, "add"))
    t5 = g.mul(K2, WK2)
    t6 = g.mul(A2, WA2)
    kk = g.mul(K2, K3)
    ae = g.mul(A2, E)
    M23e = g.add(g.sub(g.sub(M13, t5), t6), g.cmul(M45, g.add(kk, ae)))

    # M column 4 (joint 5); M44 is a constant
    P1 = g.sub(g.mul(K6, c234), g.mul(A6, s234))
    P2 = g.sub(g.mul(K2, c234), g.mul(A2, s234))
    P3 = g.sub(g.mul(K3, c234), g.mul(E, s234))
    M14 = g.cmul(M5 * D6, g.mul(c5, P1))
    M24 = g.sub(M14, g.cmul(M5 * D6, g.mul(c5, P2)))
    M34 = g.sub(M14, g.cmul(M5 * D6, g.mul(c5, P3)))
    M44C = M5 * D6 * D6 + ROTOR

    # direction to hand in the rotated frame
    hxr = g.add(g.mul(c1, hx), g.mul(s1, hy))
    hyr = g.sub(g.mul(s1, hx), g.mul(c1, hy))
    dx = g.sub(hxr, A6)
    dy = g.sub(hyr, B6)
    dz = g.sub(g.add(hz, g.C(-D1)), K6)
    n2 = g.add(g.add(g.sq(dx), g.sq(dy)), g.sq(dz))

    # vd = Je^T d
    vd0 = g.sub(g.mul(A6, dy), g.mul(B6, dx))
    vd1 = g.sub(g.mul(K6, dx), g.mul(A6, dz))
    vd2 = g.sub(vd1, g.sub(g.mul(K2, dx), g.mul(A2, dz)))
    vd3 = g.sub(vd1, g.sub(g.mul(K3, dx), g.mul(E, dz)))
    vd4 = g.cmul(D6, g.add(g.add(g.mul(c45, dx), g.mul(s5, dy)),
                           g.mul(s45, dz)))
    vd = [vd0, vd1, vd2, vd3, vd4]

    M = {(0, 0): M00, (0, 1): M01, (0, 2): M02, (0, 3): M03, (0, 4): M04,
         (1, 1): M11, (1, 2): M12, (1, 3): M13, (1, 4): M14,
         (2, 2): M22, (2, 3): M23e, (2, 4): M24,
         (3, 3): M33, (3, 4): M34}

    # Cholesky (5x5, last pivot has constant diagonal M44C)
    L = {}
    rinv = []
    for jc in range(5):
        if jc < 4:
            dd = M[(jc, jc)]
            for t in range(jc):
                dd = g.sub(dd, g.sq(L[(jc, t)]))
        else:
            dd = g.ts2(g.sq(L[(4, 0)]), -1.0, "mult", M44C, "add")
            for t in range(1, 4):
                dd = g.sub(dd, g.sq(L[(4, t)]))
        r = g.recip(g.sqrt_(dd))
        rinv.append(r)
        for kk2 in range(jc + 1, 5):
            a = M[(jc, kk2)]
            for t in range(jc):
                a = g.sub(a, g.mul(L[(kk2, t)], L[(jc, t)]))
            L[(kk2, jc)] = g.mul(a, r)

    # forward solve L y = vd ; s = |y|^2
    y = []
    for j in range(5):
        a = vd[j]
        for t in range(j):
            a = g.sub(a, g.mul(L[(j, t)], y[t]))
        y.append(g.mul(a, rinv[j]))
    sacc = None
    for j in range(5):
        t = g.sq(y[j])
        sacc = t if sacc is None else g.add(sacc, t)
    cost_neg = g.mul(g.cmul(-1.0, g.recip(sacc)), n2)
    return g, cost_neg


# ----------------------------------------------------------------------------
# numpy evaluation of the DAG (for validation in test.py)
# ----------------------------------------------------------------------------

def eval_numpy(g, root, chans):
    val = {}
    for n in g.nodes:
        if n.op == "const":
            val[n.id] = np.float32(n.c)
        elif n.op == "in":
            val[n.id] = chans[n.c]
        elif n.op == "add":
            val[n.id] = val[n.args[0].id] + val[n.args[1].id]
        elif n.op == "sub":
            val[n.id] = val[n.args[0].id] - val[n.args[1].id]
        elif n.op == "mul":
            val[n.id] = val[n.args[0].id] * val[n.args[1].id]
        elif n.op == "square":
            val[n.id] = val[n.args[0].id] * val[n.args[0].id]
        elif n.op == "cmul":
            val[n.id] = np.float32(n.c) * val[n.args[0].id]
        elif n.op == "cadd":
            val[n.id] = val[n.args[0].id] + np.float32(n.c)
        elif n.op == "sin":
            sc, b = n.c
            val[n.id] = np.sin(np.float32(sc) * val[n.args[0].id] + np.float32(b))
        elif n.op == "ts2":
            s1, op0, s2, op1 = n.c
            v = val[n.args[0].id]
            for s_, o_ in ((s1, op0), (s2, op1)):
                if o_ == "mult":
                    v = v * np.float32(s_)
                else:
                    v = v + np.float32(s_)
            val[n.id] = v
        elif n.op == "sqrt":
            val[n.id] = np.sqrt(val[n.args[0].id])
        elif n.op == "recip":
            val[n.id] = np.float32(1.0) / val[n.args[0].id]
        else:
            raise ValueError(n.op)
        if n.op != "const":
            val[n.id] = val[n.id].astype(np.float32)
    return val[root.id]


def ref_numpy(x):
    """Full-pipeline numpy reference using the DAG; x [B,H,26] -> [B]."""
    B, H, Cc = x.shape
    N = B * H
    flat = x.reshape(N, Cc).astype(np.float32)
    g, root = build_graph()
    chans = {i: flat[:, SRC_COLS[i]] for i in range(8)}
    cn = eval_numpy(g, root, chans)
    return cn.reshape(B, H).sum(axis=1)


# ----------------------------------------------------------------------------
# planning: STT fusion + ETF list scheduling across dve/act/gps
# ----------------------------------------------------------------------------

# pipelined per-[128,128]-op costs (TimelineSim probe)
COST = {
    ("dve", "tt"): 212.0, ("dve", "stt"): 212.0, ("dve", "ts"): 162.0,
    ("dve", "recip"): 204.0, ("dve", "reduce"): 296.0,
    ("act", "any"): 360.0,
    ("gps", "tt"): 440.0, ("gps", "ts"): 360.0,
}
XLAT = 100.0  # cross-engine semaphore latency


def classify(n):
    """Returns options = [(engine, cost), ...]. GPSIMD (Pool) supports only
    tensor_tensor and tensor_scalar; scalar_tensor_tensor is DVE-only."""
    if n.op == "sin" or n.op == "sqrt":
        return [("act", COST[("act", "any")])]
    if n.op == "recip":
        return [("dve", COST[("dve", "recip")])]
    if n.op == "square":
        return [("dve", COST[("dve", "tt")]), ("act", COST[("act", "any")]),
                ("gps", COST[("gps", "tt")])]
    if n.op in ("cadd", "cmul", "ts2"):
        # ts2 with non-(mult,add) pattern can't be an ACT Copy
        actok = True
        if n.op == "ts2" and (n.c[1], n.c[3]) != ("mult", "add"):
            actok = False
        opts = [("dve", COST[("dve", "ts")])]
        if actok:
            opts.append(("act", COST[("act", "any")]))
        opts.append(("gps", COST[("gps", "ts")]))
        return opts
    if n.op in ("add", "sub", "mul"):
        if isinstance(n.c, tuple) and n.c and n.c[0] == "stt_cmul":
            return [("dve", COST[("dve", "stt")])]
        return [("dve", COST[("dve", "tt")]), ("gps", COST[("gps", "tt")])]
    if n.op == "cmul_stt":  # cmul fused with mul/square arg
        return [("dve", COST[("dve", "stt")])]
    raise ValueError(n.op)


def plan(g, root):
    """STT fusion + ETF scheduling. Returns emit list ordered by virtual
    start time, with n.engine set."""
    # reachability + users
    reach = set()
    stack = [root]
    while stack:
        n = stack.pop()
        if n.id in reach:
            continue
        reach.add(n.id)
        stack.extend(n.args)
    for n in g.nodes:
        n.users = []
    order = [n for n in g.nodes if n.id in reach]
    for n in order:
        for a in n.args:
            a.users.append(n)

    # fusion: add/sub(x, cmul(c,y)) -> STT ; cmul(c, mul(x,y)/square(x)) -> STT
    for n in order:
        if n.op in ("add", "sub"):
            for k, a in enumerate(n.args):
                if a.op == "cmul" and len(a.users) == 1 and a.fused_into is None \
                        and a.args[0].fused_into is None \
                        and a.args[0].op != "const":
                    n.c = ("stt_cmul", k, a.c)
                    a.fused_into = n
                    break
        elif n.op == "cmul" and n.fused_into is None:
            a = n.args[0]
            if a.op in ("mul", "square") and len(a.users) == 1 \
                    and a.fused_into is None \
                    and all(aa.fused_into is None for aa in a.args):
                a.fused_into = n

    # effective deps of an emitted node (skipping fused producers)
    def deps(n):
        out = []
        for a in n.args:
            if a.fused_into is n:
                for aa in a.args:
                    if aa.op not in ("const", "in"):
                        out.append(aa)
            elif a.op not in ("const", "in"):
                out.append(a)
        return out

    emit_nodes = [n for n in order
                  if n.op not in ("const", "in") and n.fused_into is None]

    def opts_of(n):
        if n.op == "cmul" and n.args[0].fused_into is n:
            return classify(Expr("cmul_stt"))
        return classify(n)

    # critical-path priority (min cost per node)
    mincost = {n.id: min(c for _, c in opts_of(n)) for n in emit_nodes}
    prio = {}

    def get_prio(n):
        if n.id in prio:
            return prio[n.id]
        p = mincost[n.id] + max(
            (get_prio(u if u.fused_into is None else u.fused_into)
             for u in n.users if (u.fused_into is None or u.fused_into is not n)
             ), default=0.0)
        prio[n.id] = p
        return p

    import sys
    sys.setrecursionlimit(10000)
    for n in emit_nodes:
        n.prio = 0.0
    # compute prios iteratively in reverse topological order (nodes list is topo)
    for n in reversed(emit_nodes):
        best = 0.0
        for u in n.users:
            tgt = u.fused_into if u.fused_into is not None else u
            if tgt is n:
                continue
            if tgt.fused_into is None and tgt.op not in ("const", "in"):
                best = max(best, tgt.prio)
        n.prio = mincost[n.id] + best

    # ETF scheduling
    eng_free = {"dve": 0.0, "act": 0.0, "gps": 0.0}
    ndeps = {n.id: 0 for n in emit_nodes}
    dep_lists = {}
    for n in emit_nodes:
        dl = deps(n)
        dep_lists[n.id] = dl
        ndeps[n.id] = len([d for d in dl if d.fused_into is None or True])
    # note: deps() returns emitted producers only (fused handled inside)
    ready = [n for n in emit_nodes if ndeps[n.id] == 0]
    users_emit = {n.id: [] for n in emit_nodes}
    for n in emit_nodes:
        for d in dep_lists[n.id]:
            users_emit[d.id].append(n)

    scheduled = []
    import heapq
    # ready heap keyed by -prio
    heap = [(-n.prio, n.id, n) for n in ready]
    heapq.heapify(heap)
    pending = []  # nodes whose deps are done
    n_done = 0
    while heap:
        # pick best (node, engine) by earliest finish; tie-break priority
        best = None
        cand = []
        # examine top K candidates by priority
        K = 8
        tmp = []
        while heap and len(tmp) < K:
            tmp.append(heapq.heappop(heap))
        for negp, _, n in tmp:
            data_ready = {}
            for e in ("dve", "act", "gps"):
                dr = 0.0
                for d in dep_lists[n.id]:
                    t = d.finish + (XLAT if d.engine != e else 0.0)
                    dr = max(dr, t)
                data_ready[e] = dr
            for e, c in opts_of(n):
                st = max(eng_free[e], data_ready[e])
                fin = st + c
                cand.append((fin, -(-negp), st, e, c, negp, n))
        cand.sort(key=lambda x: (x[0], x[5]))
        fin, _, st, e, c, negp, n = cand[0]
        # push back the unchosen
        for negp2, i2, n2 in tmp:
            if n2 is not n:
                heapq.heappush(heap, (negp2, i2, n2))
        n.engine = e
        n.start = st
        n.finish = fin
        eng_free[e] = fin
        scheduled.append(n)
        for u in users_emit[n.id]:
            ndeps[u.id] -= 1
            if ndeps[u.id] == 0:
                heapq.heappush(heap, (-u.prio, u.id, u))

    scheduled.sort(key=lambda n: n.start)
    for i, n in enumerate(scheduled):
        n.order = i
    load = {e: 0.0 for e in eng_free}
    for n in scheduled:
        load[n.engine] += [c for ee, c in opts_of(n) if ee == n.engine][0]
    makespan = max(n.finish for n in scheduled)
    return scheduled, load, makespan


# ----------------------------------------------------------------------------
# bass emission
# ----------------------------------------------------------------------------

NCORES = 8
B_FULL, H, CH = 2048, 64, 26
N_PER_CORE = B_FULL * H // NCORES          # 16384
P = 128
FD = N_PER_CORE // P                        # 128
NCH = 8


def _build_bass():
    import concourse.bass as bass
    from concourse.bacc import Bacc
    import concourse.mybir as mybir
    from concourse.tile import TileContext

    f32 = mybir.dt.float32
    alu = mybir.AluOpType
    AF = mybir.ActivationFunctionType

    g, root = build_graph()
    emit, load, makespan = plan(g, root)

    nc = Bacc()
    # const APs for non-Copy activation biases
    for cv in (PI / 2,):
        t = nc.alloc_sbuf_tensor(f"constf32-{cv}", [128, 1], f32)
        nc.gpsimd.memset(t.ap(), cv)
        nc.const_aps.aps[(f32, float(cv))] = t.ap()
    nc.all_engine_barrier()
    xs = nc.dram_tensor("xs", (P, NCH * FD), f32, kind="ExternalInput")
    out = nc.dram_tensor("out", (B_FULL // NCORES,), f32, kind="ExternalOutput")

    # liveness for slot allocation
    last_use = {}
    for n in emit:
        for a in n.args:
            if a.order is not None:
                last_use[a.id] = max(last_use.get(a.id, -1), n.order)
            if a.fused_into is n:
                for aa in a.args:
                    if aa.order is not None:
                        last_use[aa.id] = max(last_use.get(aa.id, -1), n.order)
    last_use[root.id] = len(emit) + 10

    with TileContext(nc) as tc:
        with tc.tile_pool(name="vals", bufs=1) as vp:
            # three staged input groups: [q2 q3 q4] [q1 q5] [hx hy hz]
            stA = vp.tile([P, 3 * FD], f32, tag="stA", name="stA")
            stB = vp.tile([P, 2 * FD], f32, tag="stB", name="stB")
            stC = vp.tile([P, 3 * FD], f32, tag="stC", name="stC")
            nc.gpsimd.dma_start(stA[:, :], xs[:, 0:3 * FD])
            nc.gpsimd.dma_start(stB[:, :], xs[:, 3 * FD:5 * FD])
            nc.gpsimd.dma_start(stC[:, :], xs[:, 5 * FD:8 * FD])

            def chan_ap(ch):
                if ch < 3:
                    return stA[:, ch * FD:(ch + 1) * FD]
                if ch < 5:
                    return stB[:, (ch - 3) * FD:(ch - 2) * FD]
                return stC[:, (ch - 5) * FD:(ch - 4) * FD]

            from collections import deque
            free_slots = deque()
            SLACK = 64
            n_slots = [0]
            node_tile = {}

            def ap_of(n):
                if n.op == "in":
                    return chan_ap(n.c)
                return node_tile[n.id][:, :]

            def alloc(n):
                if len(free_slots) > SLACK:
                    sl = free_slots.popleft()
                else:
                    sl = n_slots[0]
                    n_slots[0] += 1
                t = vp.tile([P, FD], f32, tag=f"s{sl}", name=f"v{n.id}", bufs=2)
                n.slot = sl
                node_tile[n.id] = t
                return t

            by_last = {}
            for nid, lu in last_use.items():
                by_last.setdefault(lu, []).append(nid)

            eng = {"dve": nc.vector, "act": nc.scalar, "gps": nc.gpsimd}
            ALU_OF = {"add": alu.add, "sub": alu.subtract, "mul": alu.mult}

            def emit_tt(e, engname, ot, a, b, op):
                e.tensor_tensor(ot, a, b, op)

            for n in emit:
                ot = alloc(n)[:, :]
                e = eng[n.engine]
                en = n.engine
                if n.op == "sin":
                    sc, b = n.c
                    nc.scalar.activation(ot, ap_of(n.args[0]), AF.Sin,
                                         bias=float(b), scale=float(sc))
                elif n.op == "sqrt":
                    nc.scalar.activation(ot, ap_of(n.args[0]), AF.Sqrt)
                elif n.op == "recip":
                    nc.vector.reciprocal_approx_fast(out=ot, in_=ap_of(n.args[0]))
                elif n.op == "square":
                    if en == "act":
                        nc.scalar.activation(ot, ap_of(n.args[0]), AF.Square)
                    else:
                        a = ap_of(n.args[0])
                        emit_tt(e, en, ot, a, a, alu.mult)
                elif n.op == "cadd":
                    if en == "act":
                        nc.scalar.activation(ot, ap_of(n.args[0]), AF.Copy,
                                             bias=float(n.c), scale=1.0)
                    else:
                        e.tensor_scalar_add(ot, ap_of(n.args[0]), float(n.c))
                elif n.op == "ts2":
                    s1, op0, s2, op1 = n.c
                    if en == "act":
                        nc.scalar.activation(ot, ap_of(n.args[0]), AF.Copy,
                                             bias=float(s2), scale=float(s1))
                    else:
                        e.tensor_scalar(ot, ap_of(n.args[0]), float(s1), float(s2),
                                        getattr(alu, op0), getattr(alu, op1))
                elif n.op == "cmul":
                    a = n.args[0]
                    if a.fused_into is n:
                        if a.op == "square":
                            x = yv = a.args[0]
                        else:
                            x, yv = a.args
                        e.scalar_tensor_tensor(ot, ap_of(x), float(n.c),
                                               ap_of(yv), alu.mult, alu.mult)
                    elif en == "act":
                        nc.scalar.activation(ot, ap_of(n.args[0]), AF.Copy,
                                             bias=0.0, scale=float(n.c))
                    else:
                        e.tensor_scalar_mul(ot, ap_of(n.args[0]), float(n.c))
                elif n.op in ("add", "sub"):
                    if isinstance(n.c, tuple) and n.c and n.c[0] == "stt_cmul":
                        _, k, cval = n.c
                        cm = n.args[k]
                        other = n.args[1 - k]
                        x = cm.args[0]
                        if n.op == "add":
                            e.scalar_tensor_tensor(ot, ap_of(x), float(cval),
                                                   ap_of(other), alu.mult, alu.add)
                        else:
                            if k == 1:
                                e.scalar_tensor_tensor(ot, ap_of(x), float(-cval),
                                                       ap_of(other), alu.mult,
                                                       alu.add)
                            else:
                                e.scalar_tensor_tensor(ot, ap_of(x), float(cval),
                                                       ap_of(other), alu.mult,
                                                       alu.subtract)
                    else:
                        emit_tt(e, en, ot, ap_of(n.args[0]), ap_of(n.args[1]),
                                ALU_OF[n.op])
                elif n.op == "mul":
                    emit_tt(e, en, ot, ap_of(n.args[0]), ap_of(n.args[1]),
                            alu.mult)
                else:
                    raise ValueError(n.op)

                for nid in by_last.get(n.order, []):
                    nd = g.nodes[nid]
                    if nd.slot is not None and nd.id != root.id:
                        free_slots.append(nd.slot)
                        nd.slot = None

            # epilogue: per-b sums (64-sample segments)
            osum = vp.tile([P, 2], f32, tag="osum", bufs=2)
            croot = node_tile[root.id]
            nc.vector.tensor_reduce(osum[:, 0:1], croot[:, 0:64],
                                    mybir.AxisListType.X, alu.add)
            nc.vector.tensor_reduce(osum[:, 1:2], croot[:, 64:128],
                                    mybir.AxisListType.X, alu.add)
            nc.gpsimd.dma_start(out.rearrange("(p j) -> p j", p=P), osum[:, :])

    nc.compile()
    return nc, len(emit), load, makespan


_CACHE = {}


def kernel(x, cond, time):
    from concourse.bass_utils import run_bass_kernel_spmd

    if "nc" not in _CACHE:
        nc, n_ops, load, makespan = _build_bass()
        _CACHE["nc"] = nc
        _CACHE["stats"] = (n_ops, load, makespan)
    nc = _CACHE["nc"]

    xf = np.ascontiguousarray(x, dtype=np.float32).reshape(B_FULL * H, CH)
    sel = xf[:, SRC_COLS]                                   # [131072, 8]
    in_maps = []
    for k in range(NCORES):
        shard = sel[k * N_PER_CORE:(k + 1) * N_PER_CORE]    # [16384, 8]
        # [128 p, 128 q, 8 c] -> [128, 8, 128] channel-major free layout
        arr = shard.reshape(P, FD, NCH).transpose(0, 2, 1)
        in_maps.append({"xs": np.ascontiguousarray(arr).reshape(P, NCH * FD)})
    res = run_bass_kernel_spmd(nc, in_maps, core_ids=list(range(NCORES)))
    _CACHE["exec_time_ns"] = res.exec_time_ns
    _CACHE["trace"] = res.instructions_and_trace
    outs = [res.results[k]["out"] for k in range(NCORES)]
    return np.concatenate(outs).astype(np.float32)


if __name__ == "__main__":
    g, root = build_graph()
    emit, load, makespan = plan(g, root)
    from collections import Counter
    print("emitted ops:", len(emit))
    print(Counter((n.engine, n.op) for n in emit))
    print("load est (us):", {k: v / 1000 for k, v in load.items()})
    print("virtual makespan (us):", makespan / 1000)
